# revision 6
# baseline (speedup 1.0000x reference)
"""GIN 2-layer GNN message passing on 8 TRN2 NeuronCores.

Strategy (dst-sharded graph parallel):
  - Nodes are sharded contiguously across 8 cores (6250 per core).
  - Edges are partitioned by destination node and sorted into 128-node
    destination chunks on the host. Per chunk, edges are processed in
    tiles of 128: indirect-DMA gather of source features (fp16 rows)
    followed by a one-hot matmul (S.T trick) that scatter-adds into a
    PSUM accumulator holding aggT [feat x chunk_nodes].
  - The full x (fp16) is replicated to every core, so layer 1 needs no
    communication. Layer-1 output h1 (fp16 rows) is AllGathered so that
    layer 2 can gather any source node's features.
  - The GIN MLPs run in transposed activation layout [feat x nodes];
    log_softmax / softmax need per-node (free-axis) reductions, so z is
    transposed back to row layout via the TensorEngine before softmax.
"""

import numpy as np

N = 50000
E = 800000
D = 128
DO = 64
NCORES = 8
NPC = N // NCORES  # 6250
CH = 128  # dst-chunk size (= PSUM free dim of the aggregation tile)
NCH = (NPC + CH - 1) // CH  # 49
NPAD = NCH * CH  # 6272

_CACHE = {}


def _prep_edges(edge_index):
    """Partition/sort edges by destination, pad to uniform tile counts.

    Returns srcT [cores,128,T] int32, dstT [cores,128,T] float32, K.
    Tile t of chunk h on core c covers column h*K+t; row p is edge p of
    that tile. dstT holds the chunk-local dst id (0..127) or CH (=128)
    for padding edges, which the one-hot build maps to a zero column.
    """
    src = np.asarray(edge_index[0]).astype(np.int64)
    dst = np.asarray(edge_index[1]).astype(np.int64)
    order = np.argsort(dst, kind="stable")
    src_s = src[order].astype(np.int32)
    dst_s = dst[order].astype(np.int64)

    core = dst_s // NPC
    ldst = dst_s - core * NPC
    chunk = ldst // CH
    gchunk = (core * NCH + chunk).astype(np.int64)
    cnts = np.bincount(gchunk, minlength=NCORES * NCH)
    K = int(np.ceil(cnts.max() / 128.0))
    starts = np.concatenate([[0], np.cumsum(cnts)])
    pos = np.arange(len(dst_s)) - starts[gchunk]
    dstloc = (ldst - chunk * CH).astype(np.float32)

    srcA = np.zeros((NCORES, NCH, K * 128), np.int32)
    dstA = np.full((NCORES, NCH, K * 128), float(CH), np.float32)
    srcA[core, chunk, pos] = src_s
    dstA[core, chunk, pos] = dstloc

    srcT = np.ascontiguousarray(srcA.reshape(NCORES, NCH * K, 128).transpose(0, 2, 1))
    dstT = np.ascontiguousarray(dstA.reshape(NCORES, NCH * K, 128).transpose(0, 2, 1))
    return srcT, dstT, K


def _build_program(K):
    from contextlib import ExitStack

    import concourse.tile as tile
    from concourse import bacc, bass, mybir

    f32 = mybir.dt.float32
    f16 = mybir.dt.float16
    i32 = mybir.dt.int32
    T = NCH * K

    nc = bacc.Bacc(
        "TRN2", target_bir_lowering=False, debug=False, num_devices=NCORES
    )

    # --- DRAM tensors ---
    xg = nc.dram_tensor("xg", [N, D], f16, kind="ExternalInput").ap()
    xT = nc.dram_tensor("xT", [D, NPAD], f32, kind="ExternalInput").ap()
    srcT = nc.dram_tensor("srcT", [128, T], i32, kind="ExternalInput").ap()
    dstT = nc.dram_tensor("dstT", [128, T], f32, kind="ExternalInput").ap()
    w = {}
    for name, shape in [
        ("w1a", [D, D]), ("w2a", [D, D]), ("w1b", [D, D]), ("w2b", [D, D]),
        ("wo", [D, DO]),
        ("b1a", [D, 1]), ("b2a", [D, 1]), ("b1b", [D, 1]), ("b2b", [D, 1]),
        ("bo", [DO, 1]),
        ("iota", [128, 128]), ("ident", [128, 128]),
    ]:
        w[name] = nc.dram_tensor(name, shape, f32, kind="ExternalInput").ap()
    h1own = nc.dram_tensor("h1own", [NPC, D], f16, kind="Internal").ap()
    h1all = nc.dram_tensor(
        "h1all", [N, D], f16, kind="Internal", addr_space="Shared"
    ).ap()
    out = nc.dram_tensor("out", [NPC, DO], f32, kind="ExternalOutput").ap()

    with tile.TileContext(nc) as tc, ExitStack() as ctx:
        consts = ctx.enter_context(tc.tile_pool(name="consts", bufs=1))
        gpool = ctx.enter_context(tc.tile_pool(name="gather", bufs=8))
        spool = ctx.enter_context(tc.tile_pool(name="sel", bufs=6))
        apsum = ctx.enter_context(tc.tile_pool(name="apsum", bufs=2, space="PSUM"))
        mpsum = ctx.enter_context(tc.tile_pool(name="mpsum", bufs=2, space="PSUM"))
        tpsum = ctx.enter_context(tc.tile_pool(name="tpsum", bufs=2, space="PSUM"))
        hpool = ctx.enter_context(tc.tile_pool(name="hT", bufs=3))
        zpool = ctx.enter_context(tc.tile_pool(name="z", bufs=6))
        rpool = ctx.enter_context(tc.tile_pool(name="rows", bufs=6))
        small = ctx.enter_context(tc.tile_pool(name="small", bufs=12))

        # --- load constants (unique tags: each persists for the whole kernel) ---
        def load_const(ap, shape, dtype, tag):
            t = consts.tile(shape, dtype, tag=tag)
            nc.sync.dma_start(t[:], ap[:])
            return t

        xT_sb = load_const(xT, [128, NPAD], f32, "c_xT")
        srcT_sb = load_const(srcT, [128, T], i32, "c_src")
        dstT_sb = load_const(dstT, [128, T], f32, "c_dst")
        wsb = {k: load_const(v, v.shape, f32, "c_" + k) for k, v in w.items()}
        iota_sb = wsb["iota"]
        ident_sb = wsb["ident"]

        # persistent transposed layer-1 output (self term for layer 2)
        h1T_sb = consts.tile([128, NPAD], f32, tag="c_h1T")

        def gin_layer(layer, table_ap, w1_sb, b1_sb, w2_sb, b2_sb):
            """One GIN layer. Returns nothing; writes h_next via callbacks."""
            for h in range(NCH):
                agg = apsum.tile([128, CH], f32, tag="agg")
                for t in range(K):
                    ti = h * K + t
                    g = gpool.tile([128, D], f16, tag="g")
                    nc.gpsimd.indirect_dma_start(
                        out=g[:],
                        out_offset=None,
                        in_=table_ap[:],
                        in_offset=bass.IndirectOffsetOnAxis(
                            ap=srcT_sb[:, ti : ti + 1], axis=0
                        ),
                    )
                    s = spool.tile([128, CH], f16, tag="s")
                    nc.vector.tensor_tensor(
                        out=s[:],
                        in0=dstT_sb[:, ti : ti + 1].to_broadcast([128, CH]),
                        in1=iota_sb[:],
                        op=mybir.AluOpType.is_equal,
                    )
                    # aggT[feat, chunk_nodes] += G.T @ S
                    nc.tensor.matmul(
                        out=agg[:],
                        lhsT=g[:],
                        rhs=s[:],
                        start=(t == 0),
                        stop=(t == K - 1),
                    )
                # hT = xT_chunk + aggT  (GIN: (1+eps)*x + agg, eps=0)
                hT = hpool.tile([128, CH], f32, tag="hT")
                if layer == 0:
                    self_sb = xT_sb
                else:
                    self_sb = h1T_sb
                nc.vector.tensor_add(
                    hT[:], self_sb[:, h * CH : (h + 1) * CH], agg[:]
                )
                # z1T = relu(w1.T @ hT + b1)
                z1p = mpsum.tile([128, CH], f32, tag="mm")
                nc.tensor.matmul(out=z1p[:], lhsT=w1_sb[:], rhs=hT[:], start=True, stop=True)
                z1 = zpool.tile([128, CH], f32, tag="z1")
                nc.scalar.activation(
                    z1[:], z1p[:], mybir.ActivationFunctionType.Relu, bias=b1_sb[:, :1]
                )
                # z2T = w2.T @ z1T + b2
                z2p = mpsum.tile([128, CH], f32, tag="mm")
                nc.tensor.matmul(out=z2p[:], lhsT=w2_sb[:], rhs=z1[:], start=True, stop=True)
                z2 = zpool.tile([128, CH], f32, tag="z2")
                nc.vector.tensor_add(
                    z2[:], z2p[:], b2_sb[:, :1].to_broadcast([128, CH])
                )
                # rows = z2.T  [nodes x feat]
                zr = tpsum.tile([128, 128], f32, tag="tp")
                nc.tensor.transpose(out=zr[:], in_=z2[:], identity=ident_sb[:])
                # log_softmax along free axis
                negm = small.tile([128, 1], f32, tag="negm")
                nc.vector.tensor_reduce(
                    negm[:], zr[:], axis=mybir.AxisListType.X,
                    op=mybir.AluOpType.max, negate=True,
                )
                e = rpool.tile([128, D], f32, tag="e")
                ssum = small.tile([128, 1], f32, tag="ssum")
                nc.scalar.activation(
                    e[:], zr[:], mybir.ActivationFunctionType.Exp,
                    bias=negm[:, :1], accum_out=ssum[:, :1],
                )
                lse = small.tile([128, 1], f32, tag="lse")
                nc.scalar.activation(
                    lse[:], ssum[:], mybir.ActivationFunctionType.Ln
                )
                shift = small.tile([128, 1], f32, tag="shift")
                nc.vector.tensor_tensor(
                    shift[:], negm[:], lse[:], op=mybir.AluOpType.subtract
                )
                hr = rpool.tile([128, D], f32, tag="hr")
                nc.vector.tensor_tensor(
                    hr[:], zr[:], shift[:, :1].to_broadcast([128, D]),
                    op=mybir.AluOpType.add,
                )
                rows = NPC - h * CH if h == NCH - 1 else CH
                if layer == 0:
                    # fp16 copy of h1 rows for the gather table
                    hf = rpool.tile([128, D], f16, tag="hf")
                    nc.vector.tensor_copy(hf[:], hr[:])
                    nc.sync.dma_start(
                        h1own[h * CH : h * CH + rows, :], hf[:rows, :]
                    )
                    # transposed copy for layer-2 self term
                    h1Tp = tpsum.tile([128, 128], f32, tag="tp")
                    nc.tensor.transpose(out=h1Tp[:], in_=hr[:], identity=ident_sb[:])
                    nc.vector.tensor_copy(
                        h1T_sb[:, h * CH : (h + 1) * CH], h1Tp[:]
                    )
                else:
                    # output head: zoT = wo.T @ h2T + bo ; softmax rows
                    h2T = hpool.tile([128, CH], f32, tag="h2T")
                    # transpose hr back to [feat x nodes]
                    h2Tp = tpsum.tile([128, 128], f32, tag="tp")
                    nc.tensor.transpose(out=h2Tp[:], in_=hr[:], identity=ident_sb[:])
                    nc.vector.tensor_copy(h2T[:], h2Tp[:])
                    zop = mpsum.tile([DO, CH], f32, tag="mm")
                    nc.tensor.matmul(
                        out=zop[:], lhsT=wsb["wo"][:], rhs=h2T[:], start=True, stop=True
                    )
                    zo = zpool.tile([DO, CH], f32, tag="zo")
                    nc.vector.tensor_add(
                        zo[:], zop[:], wsb["bo"][:, :1].to_broadcast([DO, CH])
                    )
                    zor = tpsum.tile([128, DO], f32, tag="tp")
                    nc.tensor.transpose(
                        out=zor[:], in_=zo[:], identity=ident_sb[:DO, :DO]
                    )
                    negm2 = small.tile([128, 1], f32, tag="negm2")
                    nc.vector.tensor_reduce(
                        negm2[:], zor[:], axis=mybir.AxisListType.X,
                        op=mybir.AluOpType.max, negate=True,
                    )
                    e2 = rpool.tile([128, DO], f32, tag="e2")
                    ssum2 = small.tile([128, 1], f32, tag="ssum2")
                    nc.scalar.activation(
                        e2[:], zor[:], mybir.ActivationFunctionType.Exp,
                        bias=negm2[:, :1], accum_out=ssum2[:, :1],
                    )
                    inv = small.tile([128, 1], f32, tag="inv")
                    nc.vector.reciprocal(inv[:], ssum2[:])
                    outr = rpool.tile([128, DO], f32, tag="outr")
                    nc.vector.tensor_tensor(
                        outr[:], e2[:], inv[:, :1].to_broadcast([128, DO]),
                        op=mybir.AluOpType.mult,
                    )
                    nc.sync.dma_start(
                        out[h * CH : h * CH + rows, :], outr[:rows, :]
                    )

        gin_layer(0, xg, wsb["w1a"], wsb["b1a"], wsb["w2a"], wsb["b2a"])
        nc.gpsimd.collective_compute(
            "AllGather",
            mybir.AluOpType.bypass,
            replica_groups=[list(range(NCORES))],
            ins=[h1own[:]],
            outs=[h1all[:]],
        )
        gin_layer(1, h1all, wsb["w1b"], wsb["b1b"], wsb["w2b"], wsb["b2b"])

    nc.compile()
    return nc


def kernel(**inputs):
    x = np.asarray(inputs["x"], np.float32)
    edge_index = np.asarray(inputs["edge_index"])

    srcT, dstT, K = _prep_edges(edge_index)

    if "nc" not in _CACHE or _CACHE.get("K") != K:
        _CACHE["nc"] = _build_program(K)
        _CACHE["K"] = K
    nc = _CACHE["nc"]

    xg = x.astype(np.float16)
    iota = np.broadcast_to(np.arange(128, dtype=np.float32), (128, 128)).copy()
    ident = np.eye(128, dtype=np.float32)

    in_maps = []
    for c in range(NCORES):
        xTc = np.zeros((D, NPAD), np.float32)
        xTc[:, :NPC] = x[c * NPC : (c + 1) * NPC].T
        m = {
            "xg": xg,
            "xT": xTc,
            "srcT": srcT[c],
            "dstT": dstT[c],
            "w1a": np.asarray(inputs["w1_0"], np.float32),
            "w2a": np.asarray(inputs["w2_0"], np.float32),
            "w1b": np.asarray(inputs["w1_1"], np.float32),
            "w2b": np.asarray(inputs["w2_1"], np.float32),
            "wo": np.asarray(inputs["wo"], np.float32),
            "b1a": np.asarray(inputs["b1_0"], np.float32).reshape(D, 1),
            "b2a": np.asarray(inputs["b2_0"], np.float32).reshape(D, 1),
            "b1b": np.asarray(inputs["b1_1"], np.float32).reshape(D, 1),
            "b2b": np.asarray(inputs["b2_1"], np.float32).reshape(D, 1),
            "bo": np.asarray(inputs["bo"], np.float32).reshape(DO, 1),
            "iota": iota,
            "ident": ident,
        }
        in_maps.append(m)

    from concourse import bass_utils

    _CACHE["in_maps"] = in_maps
    res = bass_utils.run_bass_kernel_spmd(
        nc, in_maps, core_ids=list(range(NCORES)), **_CACHE.get("run_kwargs", {})
    )
    _CACHE["last_result"] = res
    outs = [res.results[c]["out"] for c in range(NCORES)]
    return np.concatenate(outs, axis=0)


# revision 7
# speedup vs baseline: 31.5453x; 31.5453x over previous
"""GIN 2-layer GNN message passing on 8 TRN2 NeuronCores.

Strategy (dst-sharded graph parallel):
  - Nodes are sharded contiguously across 8 cores (6250 per core).
  - Edges are partitioned by destination node and sorted into 128-node
    destination chunks on the host. Per chunk, edges are processed in
    tiles of 128: indirect-DMA gather of source features (fp16 rows)
    followed by a one-hot matmul (S.T trick) that scatter-adds into a
    PSUM accumulator holding aggT [feat x chunk_nodes].
  - The full x (fp16) is replicated to every core, so layer 1 needs no
    communication. Layer-1 output h1 (fp16 rows) is AllGathered so that
    layer 2 can gather any source node's features.
  - The GIN MLPs run in transposed activation layout [feat x nodes];
    log_softmax / softmax need per-node (free-axis) reductions, so z is
    transposed back to row layout via the TensorEngine before softmax.
"""

import numpy as np

N = 50000
E = 800000
D = 128
DO = 64
NCORES = 8
NPC = N // NCORES  # 6250
CH = 128  # dst-chunk size (= PSUM free dim of the aggregation tile)
NCH = (NPC + CH - 1) // CH  # 49
NPAD = NCH * CH  # 6272

_CACHE = {}


def _prep_edges(edge_index):
    """Partition/sort edges by destination, pad to uniform tile counts.

    Returns srcT [cores,128,T] int32, dstT [cores,128,T] float32, K.
    Tile t of chunk h on core c covers column h*K+t; row p is edge p of
    that tile. dstT holds the chunk-local dst id (0..127) or CH (=128)
    for padding edges, which the one-hot build maps to a zero column.
    """
    src = np.asarray(edge_index[0]).astype(np.int64)
    dst = np.asarray(edge_index[1]).astype(np.int64)
    order = np.argsort(dst, kind="stable")
    src_s = src[order].astype(np.int32)
    dst_s = dst[order].astype(np.int64)

    core = dst_s // NPC
    ldst = dst_s - core * NPC
    chunk = ldst // CH
    gchunk = (core * NCH + chunk).astype(np.int64)
    cnts = np.bincount(gchunk, minlength=NCORES * NCH)
    K = int(np.ceil(cnts.max() / 128.0))
    starts = np.concatenate([[0], np.cumsum(cnts)])
    pos = np.arange(len(dst_s)) - starts[gchunk]
    dstloc = (ldst - chunk * CH).astype(np.float32)

    srcA = np.zeros((NCORES, NCH, K * 128), np.int32)
    dstA = np.full((NCORES, NCH, K * 128), float(CH), np.float32)
    srcA[core, chunk, pos] = src_s
    dstA[core, chunk, pos] = dstloc

    srcT = np.ascontiguousarray(srcA.reshape(NCORES, NCH * K, 128).transpose(0, 2, 1))
    dstT = np.ascontiguousarray(dstA.reshape(NCORES, NCH * K, 128).transpose(0, 2, 1))
    return srcT, dstT, K


def _build_program(K, loops=1):
    from contextlib import ExitStack

    import concourse.tile as tile
    from concourse import bacc, bass, mybir

    f32 = mybir.dt.float32
    f16 = mybir.dt.float16
    i32 = mybir.dt.int32
    T = NCH * K

    nc = bacc.Bacc(
        "TRN2", target_bir_lowering=False, debug=False, num_devices=NCORES
    )

    # --- DRAM tensors ---
    xg = nc.dram_tensor("xg", [N, D], f16, kind="ExternalInput").ap()
    xT = nc.dram_tensor("xT", [D, NPAD], f32, kind="ExternalInput").ap()
    srcT = nc.dram_tensor("srcT", [128, T], i32, kind="ExternalInput").ap()
    dstT = nc.dram_tensor("dstT", [128, T], f32, kind="ExternalInput").ap()
    w = {}
    for name, shape in [
        ("w1a", [D, D]), ("w2a", [D, D]), ("w1b", [D, D]), ("w2b", [D, D]),
        ("wo", [D, DO]),
        ("b1a", [D, 1]), ("b2a", [D, 1]), ("b1b", [D, 1]), ("b2b", [D, 1]),
        ("bo", [DO, 1]),
        ("iota", [128, 128]), ("ident", [128, 128]),
    ]:
        w[name] = nc.dram_tensor(name, shape, f32, kind="ExternalInput").ap()
    h1own = nc.dram_tensor("h1own", [NPC, D], f16, kind="Internal").ap()
    h1all = nc.dram_tensor(
        "h1all", [N, D], f16, kind="Internal", addr_space="Shared"
    ).ap()
    out = nc.dram_tensor("out", [NPC, DO], f32, kind="ExternalOutput").ap()

    with tile.TileContext(nc) as tc, ExitStack() as ctx:
        consts = ctx.enter_context(tc.tile_pool(name="consts", bufs=1))
        gpool = ctx.enter_context(tc.tile_pool(name="gather", bufs=8))
        spool = ctx.enter_context(tc.tile_pool(name="sel", bufs=6))
        apsum = ctx.enter_context(tc.tile_pool(name="apsum", bufs=2, space="PSUM"))
        mpsum = ctx.enter_context(tc.tile_pool(name="mpsum", bufs=2, space="PSUM"))
        tpsum = ctx.enter_context(tc.tile_pool(name="tpsum", bufs=2, space="PSUM"))
        hpool = ctx.enter_context(tc.tile_pool(name="hT", bufs=3))
        zpool = ctx.enter_context(tc.tile_pool(name="z", bufs=6))
        rpool = ctx.enter_context(tc.tile_pool(name="rows", bufs=6))
        small = ctx.enter_context(tc.tile_pool(name="small", bufs=12))

        # --- load constants (unique tags: each persists for the whole kernel) ---
        def load_const(ap, shape, dtype, tag):
            t = consts.tile(shape, dtype, tag=tag)
            nc.sync.dma_start(t[:], ap[:])
            return t

        xT_sb = load_const(xT, [128, NPAD], f32, "c_xT")
        srcT_sb = load_const(srcT, [128, T], i32, "c_src")
        dstT_sb = load_const(dstT, [128, T], f32, "c_dst")
        wsb = {k: load_const(v, v.shape, f32, "c_" + k) for k, v in w.items()}
        iota_sb = wsb["iota"]
        ident_sb = wsb["ident"]

        # persistent transposed layer-1 output (self term for layer 2)
        h1T_sb = consts.tile([128, NPAD], f32, tag="c_h1T")

        def gin_layer(layer, table_ap, w1_sb, b1_sb, w2_sb, b2_sb):
            """One GIN layer. Returns nothing; writes h_next via callbacks."""
            for h in range(NCH):
                agg = apsum.tile([128, CH], f32, tag="agg")
                for t in range(K):
                    ti = h * K + t
                    g = gpool.tile([128, D], f16, tag="g")
                    nc.gpsimd.indirect_dma_start(
                        out=g[:],
                        out_offset=None,
                        in_=table_ap[:],
                        in_offset=bass.IndirectOffsetOnAxis(
                            ap=srcT_sb[:, ti : ti + 1], axis=0
                        ),
                    )
                    s = spool.tile([128, CH], f16, tag="s")
                    nc.vector.tensor_tensor(
                        out=s[:],
                        in0=dstT_sb[:, ti : ti + 1].to_broadcast([128, CH]),
                        in1=iota_sb[:],
                        op=mybir.AluOpType.is_equal,
                    )
                    # aggT[feat, chunk_nodes] += G.T @ S
                    nc.tensor.matmul(
                        out=agg[:],
                        lhsT=g[:],
                        rhs=s[:],
                        start=(t == 0),
                        stop=(t == K - 1),
                    )
                # hT = xT_chunk + aggT  (GIN: (1+eps)*x + agg, eps=0)
                hT = hpool.tile([128, CH], f32, tag="hT")
                if layer == 0:
                    self_sb = xT_sb
                else:
                    self_sb = h1T_sb
                nc.vector.tensor_add(
                    hT[:], self_sb[:, h * CH : (h + 1) * CH], agg[:]
                )
                # z1T = relu(w1.T @ hT + b1)
                z1p = mpsum.tile([128, CH], f32, tag="mm")
                nc.tensor.matmul(out=z1p[:], lhsT=w1_sb[:], rhs=hT[:], start=True, stop=True)
                z1 = zpool.tile([128, CH], f32, tag="z1")
                nc.scalar.activation(
                    z1[:], z1p[:], mybir.ActivationFunctionType.Relu, bias=b1_sb[:, :1]
                )
                # z2T = w2.T @ z1T + b2
                z2p = mpsum.tile([128, CH], f32, tag="mm")
                nc.tensor.matmul(out=z2p[:], lhsT=w2_sb[:], rhs=z1[:], start=True, stop=True)
                z2 = zpool.tile([128, CH], f32, tag="z2")
                nc.vector.tensor_add(
                    z2[:], z2p[:], b2_sb[:, :1].to_broadcast([128, CH])
                )
                # rows = z2.T  [nodes x feat]
                zr = tpsum.tile([128, 128], f32, tag="tp")
                nc.tensor.transpose(out=zr[:], in_=z2[:], identity=ident_sb[:])
                # log_softmax along free axis
                negm = small.tile([128, 1], f32, tag="negm")
                nc.vector.tensor_reduce(
                    negm[:], zr[:], axis=mybir.AxisListType.X,
                    op=mybir.AluOpType.max, negate=True,
                )
                e = rpool.tile([128, D], f32, tag="e")
                ssum = small.tile([128, 1], f32, tag="ssum")
                nc.scalar.activation(
                    e[:], zr[:], mybir.ActivationFunctionType.Exp,
                    bias=negm[:, :1], accum_out=ssum[:, :1],
                )
                lse = small.tile([128, 1], f32, tag="lse")
                nc.scalar.activation(
                    lse[:], ssum[:], mybir.ActivationFunctionType.Ln
                )
                shift = small.tile([128, 1], f32, tag="shift")
                nc.vector.tensor_tensor(
                    shift[:], negm[:], lse[:], op=mybir.AluOpType.subtract
                )
                hr = rpool.tile([128, D], f32, tag="hr")
                nc.vector.tensor_tensor(
                    hr[:], zr[:], shift[:, :1].to_broadcast([128, D]),
                    op=mybir.AluOpType.add,
                )
                rows = NPC - h * CH if h == NCH - 1 else CH
                if layer == 0:
                    # fp16 copy of h1 rows for the gather table
                    hf = rpool.tile([128, D], f16, tag="hf")
                    nc.vector.tensor_copy(hf[:], hr[:])
                    nc.sync.dma_start(
                        h1own[h * CH : h * CH + rows, :], hf[:rows, :]
                    )
                    # transposed copy for layer-2 self term
                    h1Tp = tpsum.tile([128, 128], f32, tag="tp")
                    nc.tensor.transpose(out=h1Tp[:], in_=hr[:], identity=ident_sb[:])
                    nc.vector.tensor_copy(
                        h1T_sb[:, h * CH : (h + 1) * CH], h1Tp[:]
                    )
                else:
                    # output head: zoT = wo.T @ h2T + bo ; softmax rows
                    h2T = hpool.tile([128, CH], f32, tag="h2T")
                    # transpose hr back to [feat x nodes]
                    h2Tp = tpsum.tile([128, 128], f32, tag="tp")
                    nc.tensor.transpose(out=h2Tp[:], in_=hr[:], identity=ident_sb[:])
                    nc.vector.tensor_copy(h2T[:], h2Tp[:])
                    zop = mpsum.tile([DO, CH], f32, tag="mm")
                    nc.tensor.matmul(
                        out=zop[:], lhsT=wsb["wo"][:], rhs=h2T[:], start=True, stop=True
                    )
                    zo = zpool.tile([DO, CH], f32, tag="zo")
                    nc.vector.tensor_add(
                        zo[:], zop[:], wsb["bo"][:, :1].to_broadcast([DO, CH])
                    )
                    zor = tpsum.tile([128, DO], f32, tag="tp")
                    nc.tensor.transpose(
                        out=zor[:], in_=zo[:], identity=ident_sb[:DO, :DO]
                    )
                    negm2 = small.tile([128, 1], f32, tag="negm2")
                    nc.vector.tensor_reduce(
                        negm2[:], zor[:], axis=mybir.AxisListType.X,
                        op=mybir.AluOpType.max, negate=True,
                    )
                    e2 = rpool.tile([128, DO], f32, tag="e2")
                    ssum2 = small.tile([128, 1], f32, tag="ssum2")
                    nc.scalar.activation(
                        e2[:], zor[:], mybir.ActivationFunctionType.Exp,
                        bias=negm2[:, :1], accum_out=ssum2[:, :1],
                    )
                    inv = small.tile([128, 1], f32, tag="inv")
                    nc.vector.reciprocal(inv[:], ssum2[:])
                    outr = rpool.tile([128, DO], f32, tag="outr")
                    nc.vector.tensor_tensor(
                        outr[:], e2[:], inv[:, :1].to_broadcast([128, DO]),
                        op=mybir.AluOpType.mult,
                    )
                    nc.sync.dma_start(
                        out[h * CH : h * CH + rows, :], outr[:rows, :]
                    )

        for _loop in range(loops):
            gin_layer(0, xg, wsb["w1a"], wsb["b1a"], wsb["w2a"], wsb["b2a"])
            nc.gpsimd.collective_compute(
                "AllGather",
                mybir.AluOpType.bypass,
                replica_groups=[list(range(NCORES))],
                ins=[h1own[:]],
                outs=[h1all[:]],
            )
            gin_layer(1, h1all, wsb["w1b"], wsb["b1b"], wsb["w2b"], wsb["b2b"])

    nc.compile()
    return nc


def kernel(**inputs):
    x = np.asarray(inputs["x"], np.float32)
    edge_index = np.asarray(inputs["edge_index"])

    srcT, dstT, K = _prep_edges(edge_index)

    if "nc" not in _CACHE or _CACHE.get("K") != K:
        _CACHE["nc"] = _build_program(K)
        _CACHE["K"] = K
    nc = _CACHE["nc"]

    xg = x.astype(np.float16)
    iota = np.broadcast_to(np.arange(128, dtype=np.float32), (128, 128)).copy()
    ident = np.eye(128, dtype=np.float32)

    in_maps = []
    for c in range(NCORES):
        xTc = np.zeros((D, NPAD), np.float32)
        xTc[:, :NPC] = x[c * NPC : (c + 1) * NPC].T
        m = {
            "xg": xg,
            "xT": xTc,
            "srcT": srcT[c],
            "dstT": dstT[c],
            "w1a": np.asarray(inputs["w1_0"], np.float32),
            "w2a": np.asarray(inputs["w2_0"], np.float32),
            "w1b": np.asarray(inputs["w1_1"], np.float32),
            "w2b": np.asarray(inputs["w2_1"], np.float32),
            "wo": np.asarray(inputs["wo"], np.float32),
            "b1a": np.asarray(inputs["b1_0"], np.float32).reshape(D, 1),
            "b2a": np.asarray(inputs["b2_0"], np.float32).reshape(D, 1),
            "b1b": np.asarray(inputs["b1_1"], np.float32).reshape(D, 1),
            "b2b": np.asarray(inputs["b2_1"], np.float32).reshape(D, 1),
            "bo": np.asarray(inputs["bo"], np.float32).reshape(DO, 1),
            "iota": iota,
            "ident": ident,
        }
        in_maps.append(m)

    from concourse import bass_utils

    _CACHE["in_maps"] = in_maps
    res = bass_utils.run_bass_kernel_spmd(
        nc, in_maps, core_ids=list(range(NCORES)), **_CACHE.get("run_kwargs", {})
    )
    _CACHE["last_result"] = res
    outs = [res.results[c]["out"] for c in range(NCORES)]
    return np.concatenate(outs, axis=0)
